# revision 1
# baseline (speedup 1.0000x reference)
"""CustomGAT (gnn_message_passing) Trainium2 kernel — 8-core SPMD.

Strategy (dst-partitioned edge parallelism, zero output collectives):
  * Host: add self-loops, LPT-balance destination nodes into (8 cores x BPC
    blocks) of 128 slots each by in-degree, group edges by dst-block, pad
    each block's edge list to NCHUNK chunks of 128 edges. Fold attn_l/attn_r
    into the projection weights so one matmul emits [xp | al | ar] rows.
  * Device phase A (replicated on each core): projection table
    TabX[slots, 384] bf16 rows = [xp bf16 x256 | al f32 x8 | ar f32 x8 |
    pad] written to HBM scratch.
  * Device phase B (per dst-block): batched edge gathers via the GPSIMD
    dma_gather ucode (two passes, lo/hi row halves, negative int16 indices
    skip slots), attention logits from the packed f32 al/ar, leaky-relu +
    exp on ACT/DVE, scatter-add via one-hot matmuls accumulated in PSUM
    (the alpha ride-along column gives the softmax denominator), then a
    per-head normalize at block end.
  * Host: concatenate per-core output shards, inverse-permute slots.
"""

import math

import numpy as np

# ---------------------------------------------------------------- constants
H = 8
C = 32
HC = H * C  # 256
IN = 256
ROW = 384  # bf16 slots: [xp 0:256 | al f32 256:272 | ar f32 272:288 | pad]
PSROW = HC + 2 * H  # 272 fp32 projection row [xp | al | ar]
P = 128
# dma_gather runtime offset (idx * row_bytes) tops out near 2^24 bytes;
# with 768B rows that caps idx at 21845. Split the table into ranges.
RNG = 21504

USE_F32R = True  # full-rate fp32 matmuls (tf32-like) for phase A


# ---------------------------------------------------------------- tile patch
def _install_tile_patch():
    """The axon-path walrus rejects >2 sync waits on one instruction; split
    the TileContext tail-drain waits into one carrier drain per proc."""
    import concourse.tile as tile
    from concourse.vector_clock import ScopedClock, VectorClock

    if getattr(tile.TileContext, "_drain_patch_installed", False):
        return

    def _drain_and_barrier(self, tick_clock, wait_clock):
        gc = tick_clock.global_clock
        n = len(gc)
        for p in range(n):
            if gc[p] == 0:
                continue
            req = VectorClock([gc[q] if q == p else 0 for q in range(n)])
            d = self.nc.sync.drain()
            wait_clock.add_sem_waits(d.ins, ScopedClock({None: req}))
        self.nc.all_engine_barrier()
        assert self.sems is not None
        popped = self.nc._tile_sem_poison_stack.pop()
        assert popped is self._sem_poison
        self.nc.clear_and_free_semaphores(list(self.sems.allocated().values()))
        self.nc.all_engine_barrier()

    tile.TileContext._drain_and_barrier = _drain_and_barrier
    tile.TileContext._drain_patch_installed = True


# ---------------------------------------------------------------- host prep
def _idx16(vals, nchunk, _unused=None):
    """Encode row indices for dma_gather: [128, nchunk*8] int16, index k at
    [k%16 (+16*rep), k//16]."""
    enc = vals.astype(np.int64).astype(np.int16)
    a = enc.reshape(nchunk * 8, 16).T  # [16, nchunk*8]
    return np.tile(a, (8, 1))  # replicate for the 8 Q7 cores


def _preprocess(x, edge_index, W, attn_l, attn_r, n_cores):
    N = x.shape[0]

    src = np.concatenate([np.asarray(edge_index[0]), np.arange(N, dtype=np.int64)])
    dst = np.concatenate([np.asarray(edge_index[1]), np.arange(N, dtype=np.int64)])
    Etot = src.shape[0]

    bpc = math.ceil(N / (n_cores * P))  # blocks per core
    nblocks = n_cores * bpc
    slots = nblocks * P

    # LPT balance: assign nodes to blocks by descending in-degree.
    deg = np.bincount(dst, minlength=N).astype(np.int64)
    order = np.argsort(-deg, kind="stable")
    import heapq

    heap = [(0, b) for b in range(nblocks)]
    heapq.heapify(heap)
    counts = np.zeros(nblocks, dtype=np.int64)
    blk_of = np.empty(N, dtype=np.int64)
    slot_of = np.empty(N, dtype=np.int64)
    for n in order:
        load, b = heapq.heappop(heap)
        blk_of[n] = b
        slot_of[n] = counts[b]
        counts[b] += 1
        load += int(deg[n])
        if counts[b] < P:
            heapq.heappush(heap, (load, b))

    # node -> table row (= global slot id)
    row_of = blk_of * P + slot_of

    eb = blk_of[dst]
    dloc = slot_of[dst]
    ecnt = np.bincount(eb, minlength=nblocks)
    nchunk = max(1, math.ceil((ecnt.max() + 1) / P))  # +1 => >=1 pad slot
    cap = nchunk * P

    order_e = np.argsort(eb, kind="stable")
    starts = np.concatenate([[0], np.cumsum(ecnt)])
    pos = np.arange(Etot, dtype=np.int64) - starts[eb[order_e]]

    # split each block's edges by src table-row range (ucode offset limit);
    # per-range chunk groups, separately padded -> all indices valid + small.
    srow = row_of[src]
    arow_local = (blk_of[dst] % bpc) * P + slot_of[dst]  # dst row in TabAR
    nranges = max(1, math.ceil(slots / RNG))
    rid = srow // RNG
    nch = []
    for r in range(nranges):
        cnt_r = np.bincount(eb[rid == r], minlength=nblocks)
        nch.append(math.ceil(cnt_r.max() / P))
    nchunk = sum(nch)
    cap = nchunk * P
    cbase = np.concatenate([[0], np.cumsum(nch)])  # chunk base per range

    gidx = np.zeros((nblocks, cap), dtype=np.int64)
    aidx = np.zeros((nblocks, cap), dtype=np.int64)
    dlocp = np.full((nblocks, cap), 200.0, dtype=np.float32)
    for r in range(nranges):
        if nch[r] == 0:
            continue
        sel = rid == r
        order_r = np.argsort(eb[sel], kind="stable")
        e_r = np.where(sel)[0][order_r]
        cnt_r = np.bincount(eb[sel], minlength=nblocks)
        s_r = np.concatenate([[0], np.cumsum(cnt_r)])
        pos_r = cbase[r] * P + np.arange(e_r.shape[0]) - s_r[eb[e_r]]
        gidx[eb[e_r], pos_r] = srow[e_r] - r * RNG  # pads stay 0 (local row 0)
        aidx[eb[e_r], pos_r] = arow_local[e_r]
        dlocp[eb[e_r], pos_r] = dloc[e_r].astype(np.float32)

    def per_core_idx(a, n):
        out = np.empty((n_cores, bpc, P, n * 8), dtype=np.int16)
        for c in range(n_cores):
            for b in range(bpc):
                out[c, b] = _idx16(a[c * bpc + b], n, None)
        return out

    idx_arrays = {}
    for r in range(nranges):
        if nch[r]:
            idx_arrays[f"x{r}"] = per_core_idx(
                gidx[:, cbase[r] * P : cbase[r + 1] * P], nch[r])
    idx_arrays["ari"] = per_core_idx(aidx, nchunk)

    dloc_d = np.ascontiguousarray(
        dlocp.reshape(n_cores, bpc, nchunk, P).transpose(0, 1, 3, 2)
    )

    # TabAR build rows: global table row for (core-local block tb, partition p)
    arw = np.empty((n_cores, bpc, P, 1), dtype=np.int32)
    for c in range(n_cores):
        for b in range(bpc):
            arw[c, b, :, 0] = (c * bpc + b) * P + np.arange(P)

    # weights: Wcat [256, 272] = [W.T | B_l | B_r]
    W = np.asarray(W, dtype=np.float32)
    attn_l = np.asarray(attn_l, dtype=np.float32).reshape(H, C)
    attn_r = np.asarray(attn_r, dtype=np.float32).reshape(H, C)
    A_l = np.zeros((HC, H), dtype=np.float32)
    A_r = np.zeros((HC, H), dtype=np.float32)
    for h in range(H):
        A_l[h * C : (h + 1) * C, h] = attn_l[h]
        A_r[h * C : (h + 1) * C, h] = attn_r[h]
    WT = np.ascontiguousarray(W.T)  # [in, hc]
    wcat = np.concatenate([WT, WT @ A_l, WT @ A_r], axis=1)  # [256, 272]
    wcat = np.ascontiguousarray(wcat.reshape(2, P, PSROW)).astype(np.float32)

    # x tiles for phase A: [T, 2, 128(in-lane), 128(node)], permuted so that
    # xp rows come out in table-row order.
    T = slots // P
    x_slot = np.zeros((slots, IN), dtype=np.float32)
    x_slot[row_of] = np.asarray(x, dtype=np.float32)
    xt = np.ascontiguousarray(
        x_slot.reshape(T, P, 2, P).transpose(0, 2, 3, 1), dtype=np.float32)

    iota = np.tile(np.arange(P, dtype=np.float32), (P, 1))  # iota[e, d] = d

    meta = dict(
        N=N, n_cores=n_cores, bpc=bpc, nchunk=nchunk, nch=nch,
        cbase=[int(v) for v in cbase], nranges=nranges,
        T=T, slots=slots, row_of=row_of,
    )
    shared = dict(xt=xt, wcat=wcat, iota=iota)
    per_core = [
        dict(dloc=dloc_d[c], arw=arw[c],
             **{k: v[c] for k, v in idx_arrays.items()})
        for c in range(n_cores)
    ]
    return meta, shared, per_core


# ---------------------------------------------------------------- device IR
def _build_program(meta):
    import concourse.bacc as bacc
    import concourse.bass as bass
    import concourse.tile as tile
    from concourse import mybir

    _install_tile_patch()

    bpc, nchunk, T = meta["bpc"], meta["nchunk"], meta["T"]
    nch, cbase, nranges = meta["nch"], meta["cbase"], meta["nranges"]
    n_cores = meta["n_cores"]
    f32 = mybir.dt.float32
    bf16 = mybir.dt.bfloat16
    i16 = mybir.dt.int16
    Alu = mybir.AluOpType
    Act = mybir.ActivationFunctionType

    mdt = mybir.dt.float32r if USE_F32R else f32
    i32 = mybir.dt.int32

    nc = bacc.Bacc("TRN2", target_bir_lowering=False, debug=False,
                   num_devices=n_cores)
    xt_in = nc.dram_tensor("xt", [T, 2, P, P], mdt, kind="ExternalInput").ap()
    wcat_in = nc.dram_tensor("wcat", [2, P, PSROW], mdt, kind="ExternalInput").ap()
    iota_in = nc.dram_tensor("iota", [P, P], f32, kind="ExternalInput").ap()
    dloc_in = nc.dram_tensor("dloc", [bpc, P, nchunk], f32, kind="ExternalInput").ap()
    arw_in = nc.dram_tensor("arw", [bpc, P, 1], i32, kind="ExternalInput").ap()
    xr_in = {}
    for r in range(nranges):
        if nch[r]:
            xr_in[r] = nc.dram_tensor(f"x{r}", [bpc, P, nch[r] * 8], i16,
                                      kind="ExternalInput").ap()
    ari_in = nc.dram_tensor("ari", [bpc, P, nchunk * 8], i16,
                            kind="ExternalInput").ap()
    out_ex = nc.dram_tensor("out", [bpc * P, HC], f32, kind="ExternalOutput").ap()

    # phase A tile grouping (amortize DMA): largest power of two dividing T, <=8
    G8 = 8
    while T % G8:
        G8 //= 2

    with tile.TileContext(nc) as tc:
        with (
            tc.tile_pool(name="const", bufs=1) as cpool,
            tc.tile_pool(name="dram", bufs=1, space="DRAM") as dpool,
        ):
            table = dpool.tile([T * P, ROW], bf16)
            tabAR = dpool.tile([bpc * P, P], bf16)
            wc0 = cpool.tile([P, PSROW], mdt, tag="wc0")
            wc1 = cpool.tile([P, PSROW], mdt, tag="wc1")
            nc.sync.dma_start(wc0[:], wcat_in[0])
            nc.sync.dma_start(wc1[:], wcat_in[1])
            iota_t = cpool.tile([P, P], f32, tag="iota")
            nc.sync.dma_start(iota_t[:], iota_in[:])

            # ---- phase A: projection table
            with (
                tc.tile_pool(name="pa", bufs=3) as pa,
                tc.tile_pool(name="pa_ps", bufs=4, space="PSUM") as paps,
            ):
                for g in range(T // G8):
                    tiles = slice(g * G8, (g + 1) * G8)
                    ld0 = pa.tile([P, G8, P], mdt, tag="ld0")
                    ld1 = pa.tile([P, G8, P], mdt, tag="ld1")
                    nc.sync.dma_start(
                        ld0[:], xt_in[tiles, 0].rearrange("u p n -> p u n"))
                    nc.sync.dma_start(
                        ld1[:], xt_in[tiles, 1].rearrange("u p n -> p u n"))
                    sbX = pa.tile([P, G8, ROW], bf16, tag="sbX")
                    for u in range(G8):
                        ps = paps.tile([P, PSROW], f32)
                        nc.tensor.matmul(ps[:], lhsT=ld0[:, u, :],
                                         rhs=wc0[:], start=True, stop=False)
                        nc.tensor.matmul(ps[:], lhsT=ld1[:, u, :],
                                         rhs=wc1[:], start=False, stop=True)
                        nc.vector.tensor_copy(sbX[:, u, 0:HC], ps[:, 0:HC])
                        nc.vector.tensor_copy(
                            sbX[:, u, HC : HC + 32].bitcast(f32),
                            ps[:, HC : HC + 16],
                        )
                    dst = table[g * G8 * P : (g + 1) * G8 * P, :].rearrange(
                        "(u p) r -> p u r", p=P
                    )
                    nc.sync.dma_start(dst[:, :, 0 : HC + 32],
                                      sbX[:, :, 0 : HC + 32])

            # ---- phase A': compact per-core [al|ar] table for dst gathers
            with tc.tile_pool(name="par", bufs=3) as par:
                for tb in range(bpc):
                    arw_t = par.tile([P, 1], i32, tag="arw_t")
                    nc.sync.dma_start(arw_t[:], arw_in[tb])
                    rowt = par.tile([P, ROW], bf16, tag="rowt")
                    nc.gpsimd.indirect_dma_start(
                        out=rowt[:], out_offset=None, in_=table[:],
                        in_offset=bass.IndirectOffsetOnAxis(ap=arw_t[:, 0:1],
                                                            axis=0),
                    )
                    nc.sync.dma_start(tabAR[tb * P : (tb + 1) * P, :],
                                      rowt[:, HC : HC + P])

            # ---- phase B: per dst-block gather + attention + scatter
            with (
                tc.tile_pool(name="gat", bufs=3) as gp,
                tc.tile_pool(name="small", bufs=3) as sp,
                tc.tile_pool(name="ps", bufs=2, space="PSUM") as psp,
            ):
                GMAX = 8  # dma_gather tops out at ~1024 indices (128/Q7 core)

                def grouped_gather(src_ap, idx_dram_b, nch, rowe, tag):
                    tiles = []
                    for g0 in range(0, nch, GMAX):
                        gsz = min(GMAX, nch - g0)
                        it = sp.tile([P, gsz * 8], i16, tag=f"{tag}i{g0}",
                                     name=f"{tag}i{g0}")
                        nc.sync.dma_start(it[:],
                                          idx_dram_b[:, g0 * 8 : (g0 + gsz) * 8])
                        gt = gp.tile([P, gsz, rowe], bf16, tag=f"{tag}g{g0}",
                                     name=f"{tag}g{g0}")
                        nc.gpsimd.dma_gather(gt[:], src_ap, it[:], gsz * P,
                                             gsz * P, rowe)
                        tiles.append(gt)
                    return tiles

                for b in range(bpc):
                    dlc = sp.tile([P, nchunk], f32, tag="dlc")
                    nc.sync.dma_start(dlc[:], dloc_in[b])
                    Gr = {}
                    for r in range(nranges):
                        if nch[r]:
                            Gr[r] = grouped_gather(table[r * RNG :, :], xr_in[r][b],
                                                   nch[r], ROW, f"R{r}")
                    Ats = grouped_gather(tabAR[:], ari_in[b], nchunk, P, "A")
                    U = psp.tile([P, HC + H], f32)
                    for j in range(nchunk):
                        r = max(rr for rr in range(nranges)
                                if nch[rr] and cbase[rr] <= j)
                        jj = j - cbase[r]
                        Gj, jj = Gr[r][jj // GMAX], jj % GMAX
                        xpg = Gj[:, jj, 0:HC]
                        al = Gj[:, jj, HC : HC + 32].bitcast(f32)[:, 0:H]
                        Aj = Ats[j // GMAX]
                        ar = Aj[:, j % GMAX, 0:32].bitcast(f32)[:, H : 2 * H]
                        MT = sp.tile([P, HC + H], bf16, tag="MT")
                        lg = sp.tile([P, H], f32, tag="lg")
                        lg2 = sp.tile([P, H], f32, tag="lg2")
                        nc.vector.tensor_tensor(out=lg[:], in0=al, in1=ar,
                                                op=Alu.add)
                        # leaky_relu(x) = max(x, 0.2x), then exp
                        nc.scalar.activation(out=lg2[:], in_=lg[:], func=Act.Copy,
                                             scale=0.2)
                        nc.vector.tensor_tensor(out=lg2[:], in0=lg[:], in1=lg2[:],
                                                op=Alu.max)
                        nc.scalar.activation(out=MT[:, HC : HC + H], in_=lg2[:],
                                             func=Act.Exp)
                        S2 = sp.tile([P, P], bf16, tag="S2")
                        nc.vector.tensor_scalar(S2[:], iota_t[:], dlc[:, j : j + 1],
                                                None, Alu.is_equal)
                        a3 = MT[:, HC : HC + H].unsqueeze(2).to_broadcast([P, H, C])
                        nc.vector.tensor_tensor(
                            out=MT[:, 0:HC].rearrange("p (h c) -> p h c", c=C),
                            in0=xpg.rearrange("p (h c) -> p h c", c=C),
                            in1=a3, op=Alu.mult,
                        )
                        nc.tensor.matmul(U[:], lhsT=S2[:], rhs=MT[:],
                                         start=(j == 0), stop=(j == nchunk - 1))
                    den = sp.tile([P, H], f32, tag="den")
                    nc.vector.tensor_scalar(den[:], U[:, HC : HC + H], 1e-6, None,
                                            Alu.max)
                    rec = sp.tile([P, H], f32, tag="rec")
                    nc.vector.reciprocal(rec[:], den[:])
                    ob = sp.tile([P, HC], f32, tag="ob")
                    r3 = rec[:].unsqueeze(2).to_broadcast([P, H, C])
                    nc.vector.tensor_tensor(
                        out=ob[:].rearrange("p (h c) -> p h c", c=C),
                        in0=U[:, 0:HC].rearrange("p (h c) -> p h c", c=C),
                        in1=r3, op=Alu.mult,
                    )
                    nc.sync.dma_start(out_ex[b * P : (b + 1) * P, :], ob[:])
    nc.compile()
    return nc


# ---------------------------------------------------------------- runner
def _run(inputs, trace=False, n_cores=8):
    from concourse.bass_utils import run_bass_kernel_spmd

    x = np.asarray(inputs["x"])
    edge_index = np.asarray(inputs["edge_index"])
    meta, shared, per_core = _preprocess(
        x, edge_index, inputs["W"], inputs["attn_l"], inputs["attn_r"], n_cores
    )
    nc = _build_program(meta)
    in_maps = [{**shared, **pc} for pc in per_core]
    res = run_bass_kernel_spmd(nc, in_maps, list(range(n_cores)), trace=trace)
    shards = np.concatenate([res.results[c]["out"] for c in range(n_cores)], axis=0)
    out = shards[meta["row_of"]]
    return np.ascontiguousarray(out.astype(np.float32)), res, meta


def kernel(**inputs) -> np.ndarray:
    out, _, _ = _run(inputs, trace=False)
    return out



# revision 3
# speedup vs baseline: 5.2741x; 5.2741x over previous
"""CustomGAT (gnn_message_passing) Trainium2 kernel — 8-core SPMD.

Strategy (edge-streaming, zero GPSIMD gathers, zero collectives):
  * Host (index/layout work only): add self-loops, LPT-balance destination
    nodes into (8 cores x bpc blocks) of 128 slots by in-degree, group edges
    by dst block, pad each block to nchunk chunks of 128 edges. Pre-gather
    the raw input rows x[src[e]] per edge into per-block matmul-ready tiles
    (bf16, contraction-major), and build the per-chunk one-hot scatter
    matrices S2 [edge,dst] / S2T [dst,edge] host-side. Fold attn_l into the
    projection weights (columns [xp | B_l | 0.2*B_l]) and attn_r into a
    separate tiny weight (war, columns [B_r | 0.2*B_r]).
  * Device per block: one batched DMA each for x-edge rows, one-hot pack,
    own-node rows. ar per dst node via matmul; per chunk: per-edge
    projection [xp | L | 0.2L] via 3 PSUM-accumulated matmuls (the S2T
    matmul adds ar[dst] straight into the logit columns), exp on ACT over
    both scale copies at once, leaky-relu via max on GPSIMD-as-vector,
    alpha*xp on DVE, then scatter-add via one-hot matmul accumulated in
    PSUM (alpha ride-along column gives the softmax denominator); per-head
    normalize at block end.
  * Host: concatenate per-core output shards, inverse-permute slots.
"""

import math

import numpy as np

# ---------------------------------------------------------------- constants
H = 8
C = 32
HC = H * C  # 256
IN = 256
P = 128
PSROW = HC + 2 * H  # 272: [xp 0:256 | L 256:264 | 0.2L 264:272]
MTROW = HC + H  # 264: [alpha*xp | alpha]


# ---------------------------------------------------------------- tile patch
def _install_tile_patch():
    """The axon-path walrus rejects >2 sync waits on one instruction; split
    the TileContext tail-drain waits into one carrier drain per proc."""
    import concourse.tile as tile
    from concourse.vector_clock import ScopedClock, VectorClock

    if getattr(tile.TileContext, "_drain_patch_installed", False):
        return

    def _drain_and_barrier(self, tick_clock, wait_clock):
        gc = tick_clock.global_clock
        n = len(gc)
        for p in range(n):
            if gc[p] == 0:
                continue
            req = VectorClock([gc[q] if q == p else 0 for q in range(n)])
            d = self.nc.sync.drain()
            wait_clock.add_sem_waits(d.ins, ScopedClock({None: req}))
        self.nc.all_engine_barrier()
        assert self.sems is not None
        popped = self.nc._tile_sem_poison_stack.pop()
        assert popped is self._sem_poison
        self.nc.clear_and_free_semaphores(list(self.sems.allocated().values()))
        self.nc.all_engine_barrier()

    tile.TileContext._drain_and_barrier = _drain_and_barrier
    tile.TileContext._drain_patch_installed = True


# ---------------------------------------------------------------- host prep
def _preprocess(x, edge_index, W, attn_l, attn_r, n_cores):
    from ml_dtypes import bfloat16

    N = x.shape[0]
    x = np.asarray(x, dtype=np.float32)

    src = np.concatenate([np.asarray(edge_index[0]), np.arange(N, dtype=np.int64)])
    dst = np.concatenate([np.asarray(edge_index[1]), np.arange(N, dtype=np.int64)])
    Etot = src.shape[0]

    bpc = math.ceil(N / (n_cores * P))  # blocks per core
    nblocks = n_cores * bpc
    slots = nblocks * P

    # LPT balance: assign nodes to blocks by descending in-degree.
    deg = np.bincount(dst, minlength=N).astype(np.int64)
    order = np.argsort(-deg, kind="stable")
    import heapq

    heap = [(0, b) for b in range(nblocks)]
    heapq.heapify(heap)
    counts = np.zeros(nblocks, dtype=np.int64)
    blk_of = np.empty(N, dtype=np.int64)
    slot_of = np.empty(N, dtype=np.int64)
    for n in order:
        load, b = heapq.heappop(heap)
        blk_of[n] = b
        slot_of[n] = counts[b]
        counts[b] += 1
        load += int(deg[n])
        if counts[b] < P:
            heapq.heappush(heap, (load, b))

    row_of = blk_of * P + slot_of  # node -> global slot id

    # group edges by dst block
    eb = blk_of[dst]
    dloc = slot_of[dst]
    ecnt = np.bincount(eb, minlength=nblocks)
    nchunk = math.ceil(ecnt.max() / P)
    cap = nchunk * P

    order_e = np.argsort(eb, kind="stable")
    starts = np.concatenate([[0], np.cumsum(ecnt)])
    pos = np.arange(Etot, dtype=np.int64) - starts[eb[order_e]]

    # padded per-block edge tables (pad: src slot irrelevant -> x row 0 but
    # one-hot rows/cols are all-zero so pads contribute nothing)
    gsrc = np.zeros((nblocks, cap), dtype=np.int64)
    gdl = np.full((nblocks, cap), 255, dtype=np.int64)  # 255 => no one-hot hit
    e_sorted = order_e
    gsrc[eb[e_sorted], pos] = src[e_sorted]
    gdl[eb[e_sorted], pos] = dloc[e_sorted]
    valid = np.zeros((nblocks, cap), dtype=bool)
    valid[eb[e_sorted], pos] = True

    # ---- xe: per-edge x rows, contraction-major  [nblocks, 128, nchunk*256]
    xs = x[gsrc.reshape(-1)].astype(bfloat16)  # [nblocks*cap, 256]
    xs[~valid.reshape(-1)] = 0
    xs = xs.reshape(nblocks, nchunk, P, 2, P)  # [tb, j, k(edge), s, p(in)]
    xe = np.ascontiguousarray(xs.transpose(0, 4, 1, 3, 2)).reshape(
        nblocks, P, nchunk * 2 * P
    )
    del xs

    # ---- s2: one-hot pack [nblocks, 128, nchunk*256]:
    #   [:, e, j*256 + d]      = S2[e, d]   (edge-partition)
    #   [:, d, j*256 + 128+e]  = S2T[d, e]  (dst-partition)
    oh = (
        gdl.reshape(nblocks, nchunk, P)[:, :, :, None]
        == np.arange(P, dtype=np.int64)[None, None, None, :]
    ).astype(bfloat16)  # [tb, j, e, d]
    a_ = oh.transpose(0, 2, 1, 3)  # [tb, e, j, d]
    b_ = oh.transpose(0, 3, 1, 2)  # [tb, d, j, e]
    s2 = np.ascontiguousarray(
        np.stack([a_, b_], axis=3).reshape(nblocks, P, nchunk * 2 * P)
    )
    del oh, a_, b_

    # ---- xo: own-node x rows, contraction-major [nblocks, 128, 256]
    x_slot = np.zeros((slots, IN), dtype=np.float32)
    x_slot[row_of] = x
    xo = np.ascontiguousarray(
        x_slot.reshape(nblocks, P, 2, P).transpose(0, 3, 2, 1)
    ).astype(bfloat16).reshape(nblocks, P, 2 * P)

    # ---- weights
    W = np.asarray(W, dtype=np.float32)
    al_ = np.asarray(attn_l, dtype=np.float32).reshape(H, C)
    ar_ = np.asarray(attn_r, dtype=np.float32).reshape(H, C)
    A_l = np.zeros((HC, H), dtype=np.float32)
    A_r = np.zeros((HC, H), dtype=np.float32)
    for h in range(H):
        A_l[h * C : (h + 1) * C, h] = al_[h]
        A_r[h * C : (h + 1) * C, h] = ar_[h]
    WT = np.ascontiguousarray(W.T)  # [256 in, 256 hc]
    B_l = WT @ A_l  # [256, 8]
    B_r = WT @ A_r
    wcat = np.concatenate([WT, B_l, 0.2 * B_l], axis=1)  # [256, 272]
    wcat = np.ascontiguousarray(wcat.reshape(2, P, PSROW)).astype(bfloat16)
    war = np.concatenate([B_r, 0.2 * B_r], axis=1)  # [256, 16]
    war = np.ascontiguousarray(war.reshape(2, P, 2 * H)).astype(bfloat16)

    meta = dict(N=N, n_cores=n_cores, bpc=bpc, nchunk=nchunk, slots=slots,
                row_of=row_of)
    shared = dict(wcat=wcat, war=war)
    per_core = [
        dict(
            xe=xe[c * bpc : (c + 1) * bpc],
            s2=s2[c * bpc : (c + 1) * bpc],
            xo=xo[c * bpc : (c + 1) * bpc],
        )
        for c in range(n_cores)
    ]
    return meta, shared, per_core


# ---------------------------------------------------------------- device IR
def _build_program(meta):
    import concourse.bacc as bacc
    import concourse.tile as tile
    from concourse import mybir

    _install_tile_patch()

    bpc, nchunk = meta["bpc"], meta["nchunk"]
    n_cores = meta["n_cores"]
    f32 = mybir.dt.float32
    bf16 = mybir.dt.bfloat16
    Alu = mybir.AluOpType
    Act = mybir.ActivationFunctionType

    nc = bacc.Bacc("TRN2", target_bir_lowering=False, debug=False,
                   num_devices=n_cores)
    xe_in = nc.dram_tensor("xe", [bpc, P, nchunk * 2 * P], bf16,
                           kind="ExternalInput").ap()
    s2_in = nc.dram_tensor("s2", [bpc, P, nchunk * 2 * P], bf16,
                           kind="ExternalInput").ap()
    xo_in = nc.dram_tensor("xo", [bpc, P, 2 * P], bf16,
                           kind="ExternalInput").ap()
    wcat_in = nc.dram_tensor("wcat", [2, P, PSROW], bf16,
                             kind="ExternalInput").ap()
    war_in = nc.dram_tensor("war", [2, P, 2 * H], bf16,
                            kind="ExternalInput").ap()
    out_ex = nc.dram_tensor("out", [bpc * P, HC], f32, kind="ExternalOutput").ap()

    with tile.TileContext(nc) as tc:
        with (
            tc.tile_pool(name="const", bufs=1) as cpool,
            tc.tile_pool(name="blk", bufs=2) as bp,
            tc.tile_pool(name="sm", bufs=3) as sp,
            tc.tile_pool(name="ps", bufs=3, space="PSUM") as psp,
            tc.tile_pool(name="psu", bufs=2, space="PSUM") as psu,
        ):
            wc0 = cpool.tile([P, PSROW], bf16, tag="wc0")
            wc1 = cpool.tile([P, PSROW], bf16, tag="wc1")
            wr0 = cpool.tile([P, 2 * H], bf16, tag="wr0")
            wr1 = cpool.tile([P, 2 * H], bf16, tag="wr1")
            nc.sync.dma_start(wc0[:], wcat_in[0])
            nc.sync.dma_start(wc1[:], wcat_in[1])
            nc.sync.dma_start(wr0[:], war_in[0])
            nc.sync.dma_start(wr1[:], war_in[1])

            for b in range(bpc):
                xo_t = bp.tile([P, 2 * P], bf16, tag="xo")
                nc.sync.dma_start(xo_t[:], xo_in[b])
                xe_t = bp.tile([P, nchunk, 2 * P], bf16, tag="xe")
                nc.sync.dma_start(xe_t[:], xe_in[b].rearrange(
                    "p (j q) -> p j q", q=2 * P))
                s2_t = bp.tile([P, nchunk, 2 * P], bf16, tag="s2")
                nc.sync.dma_start(s2_t[:], s2_in[b].rearrange(
                    "p (j q) -> p j q", q=2 * P))

                # ar per dst node of this block: [128, 16] = [ar | 0.2 ar]
                psar = psu.tile([P, 2 * H], f32, tag="psar")
                nc.tensor.matmul(psar[:], lhsT=xo_t[:, 0:P], rhs=wr0[:],
                                 start=True, stop=False)
                nc.tensor.matmul(psar[:], lhsT=xo_t[:, P : 2 * P], rhs=wr1[:],
                                 start=False, stop=True)
                arb = sp.tile([P, 2 * H], bf16, tag="arb")
                nc.vector.tensor_copy(arb[:], psar[:])

                U = psu.tile([P, MTROW], f32, tag="U")
                for j in range(nchunk):
                    lhs0 = xe_t[:, j, 0:P]
                    lhs1 = xe_t[:, j, P : 2 * P]
                    S2 = s2_t[:, j, 0:P]
                    S2T = s2_t[:, j, P : 2 * P]
                    PS = psp.tile([P, PSROW], f32)
                    nc.tensor.matmul(PS[:], lhsT=lhs0, rhs=wc0[:],
                                     start=True, stop=False)
                    nc.tensor.matmul(PS[:], lhsT=lhs1, rhs=wc1[:],
                                     start=False, stop=False)
                    # adds [ar | 0.2 ar] of dst into the logit columns
                    nc.tensor.matmul(PS[:, HC : HC + 2 * H], lhsT=S2T,
                                     rhs=arb[:], start=False, stop=True,
                                     skip_group_check=True)
                    T16 = sp.tile([P, 2 * H], bf16, tag="T16")
                    nc.scalar.activation(out=T16[:], in_=PS[:, HC : HC + 2 * H],
                                         func=Act.Exp)
                    MT = sp.tile([P, MTROW], bf16, tag="MT")
                    # alpha = exp(leaky_relu(L)) = max(exp(L), exp(0.2 L))
                    nc.vector.tensor_tensor(out=MT[:, HC : HC + H],
                                            in0=T16[:, 0:H], in1=T16[:, H : 2 * H],
                                            op=Alu.max)
                    a3 = MT[:, HC : HC + H].unsqueeze(2).to_broadcast([P, H, C])
                    nc.vector.tensor_tensor(
                        out=MT[:, 0:HC].rearrange("p (h c) -> p h c", c=C),
                        in0=PS[:, 0:HC].rearrange("p (h c) -> p h c", c=C),
                        in1=a3, op=Alu.mult,
                    )
                    nc.tensor.matmul(U[:], lhsT=S2, rhs=MT[:],
                                     start=(j == 0), stop=(j == nchunk - 1))

                den = sp.tile([P, H], f32, tag="den")
                nc.vector.tensor_scalar(den[:], U[:, HC : HC + H], 1e-6, None,
                                        Alu.max)
                rec = sp.tile([P, H], f32, tag="rec")
                nc.vector.reciprocal(rec[:], den[:])
                ob = sp.tile([P, HC], f32, tag="ob")
                r3 = rec[:].unsqueeze(2).to_broadcast([P, H, C])
                nc.vector.tensor_tensor(
                    out=ob[:].rearrange("p (h c) -> p h c", c=C),
                    in0=U[:, 0:HC].rearrange("p (h c) -> p h c", c=C),
                    in1=r3, op=Alu.mult,
                )
                nc.sync.dma_start(out_ex[b * P : (b + 1) * P, :], ob[:])
    nc.compile()
    return nc


# ---------------------------------------------------------------- runner
def _run(inputs, trace=False, n_cores=8):
    from concourse.bass_utils import run_bass_kernel_spmd

    x = np.asarray(inputs["x"])
    edge_index = np.asarray(inputs["edge_index"])
    meta, shared, per_core = _preprocess(
        x, edge_index, inputs["W"], inputs["attn_l"], inputs["attn_r"], n_cores
    )
    nc = _build_program(meta)
    in_maps = [{**shared, **pc} for pc in per_core]
    res = run_bass_kernel_spmd(nc, in_maps, list(range(n_cores)), trace=trace)
    shards = np.concatenate([res.results[c]["out"] for c in range(n_cores)], axis=0)
    out = shards[meta["row_of"]]
    return np.ascontiguousarray(out.astype(np.float32)), res, meta


def kernel(**inputs) -> np.ndarray:
    out, _, _ = _run(inputs, trace=False)
    return out
